# revision 1
# baseline (speedup 1.0000x reference)
"""Trainium2 Bass kernel for nn_BipartiteGraph1d (gnn_message_passing).

Reference computation (N=16384 rows, D=1024 features, L=num_layers=8):
    history[0] = x
    for i in 1..L-1:
        y = mean_j( history[j] @ m(i,j) )   j in 0..i-1, k = i-j-1
            m(i,j) = att_source[k]                    (i even, j even)
                     target_to_source * att_source[k] (i even, j odd)
                     source_to_target * att_target[k] (i odd,  j even)
                     att_target[k]                    (i odd,  j odd)
        history.append(layernorm(relu(y)))
    out = stack(history[-2:])                         (2, N, D)

Strategy (8 NeuronCores, data-parallel over rows):
  * each core gets 2048 rows, processed in 512-row blocks whose full layer
    history lives in SBUF as PE-transposed tiles hT[j] = h_j.T ([D, 512]).
  * per layer, the mean over j is accumulated directly in PSUM across all
    (j, k-chunk) contributions: 8 psum banks = 4 row-chunks x 2 dout-chunks.
  * weights stream from HBM as [128, 1024] chunks (moving operand of the
    matmul), float32r dtype -> 1 cycle/row on the PE (fp32 data, TF32-class
    rounding, ~1e-4 matmul error), on the sync-engine HWDGE ring.
  * derived matrices (elementwise products with source_to_target /
    target_to_source) are precomputed into per-matrix DRAM scratch tiles on
    the DVE, with emission interleaved so each matrix is produced just
    before the layer that first consumes it; x loads / outputs / precompute
    streams ride the scalar-engine HWDGE ring so they never queue behind
    the weight stream.
  * per layer the j's are ordered so the freshest derived matrix is
    second-to-last and the freshest history tile last - maximum slack for
    the LN->transpose pipeline and the precompute.
  * relu+layernorm run natively per-row (rows on partitions) on ACT/DVE;
    normalized output is PE-transposed back into the SBUF history.
  * zero-weight "warm" matmuls at layer boundaries keep the PE activity
    monitor from halving the clock (idle >3.4us => 1.2 GHz for >10us).
"""

import numpy as np

_CACHE = {}
_LDW_PATCHED = False


def _enable_ldw_opt():
    """Walrus ships with --enable-ldw-opt=false; our matmul stream issues
    consecutive same-weight pairs (c=0/1 share lhsT), so redundant
    LDWEIGHTS elision is pure win. Correctness is covered by the
    end-to-end error check."""
    global _LDW_PATCHED
    if _LDW_PATCHED:
        return
    import concourse.bass_utils as bu
    orig = bu.run_command

    def patched(argv, **kw):
        argv = ["--enable-ldw-opt=true" if a == "--enable-ldw-opt=false" else a
                for a in argv]
        return orig(argv, **kw)

    bu.run_command = patched
    _LDW_PATCHED = True

TAILKC = 3       # k-chunks of the last j run r-outer (boundary overlap)
WARM_TAIL = 0    # zero-matmuls right after each layer's real stream
WARM_TP = 0      # zero-matmuls after each transpose group


def _build(L, rows_per_core, D, S, block, num_devices):
    import concourse.tile as tile
    import concourse.mybir as mybir
    from concourse import bacc
    from contextlib import ExitStack

    F32R = mybir.dt.float32r
    F32 = mybir.dt.float32
    Relu = mybir.ActivationFunctionType.Relu
    Sqrt = mybir.ActivationFunctionType.Sqrt

    assert D == 1024, "layout hardcodes D=1024"
    assert rows_per_core % block == 0 and block % 128 == 0
    assert 2 <= L <= S + 1
    KC = D // 128          # contraction chunks per matrix
    RC = block // 128      # row chunks per block
    CC = D // 512          # dout chunks (psum bank width)
    NBLK = rows_per_core // block

    nc = bacc.Bacc("TRN2", target_bir_lowering=False, debug=False,
                   num_devices=num_devices)
    x_d = nc.dram_tensor("x", [rows_per_core, D], F32R, kind="ExternalInput").ap()
    s2t_d = nc.dram_tensor("source_to_target", [D, D], F32R, kind="ExternalInput").ap()
    t2s_d = nc.dram_tensor("target_to_source", [D, D], F32R, kind="ExternalInput").ap()
    As_d = nc.dram_tensor("att_source", [S, D, D], F32R, kind="ExternalInput").ap()
    At_d = nc.dram_tensor("att_target", [S, D, D], F32R, kind="ExternalInput").ap()
    id_d = nc.dram_tensor("ident", [128, 128], F32R, kind="ExternalInput").ap()
    zero_d = nc.dram_tensor("zeros", [128, 512], F32R, kind="ExternalInput").ap()
    out_d = nc.dram_tensor("out", [2, rows_per_core, D], F32R,
                           kind="ExternalOutput").ap()

    # derived matrices needed: k = i-j-1 (always < S here, so k % S == k)
    need_b = sorted({i - j - 1 for i in range(1, L) for j in range(i)
                     if i % 2 == 0 and j % 2 == 1})
    need_c = sorted({i - j - 1 for i in range(1, L) for j in range(i)
                     if i % 2 == 1 and j % 2 == 0})

    with tile.TileContext(nc) as tc, ExitStack() as ctx:
        cst = ctx.enter_context(tc.tile_pool(name="cst", bufs=1))
        hist = ctx.enter_context(tc.tile_pool(name="hist", bufs=1))
        wp = ctx.enter_context(tc.tile_pool(name="wp", bufs=6))
        zp = ctx.enter_context(tc.tile_pool(name="zp", bufs=3))
        hp = ctx.enter_context(tc.tile_pool(name="hp", bufs=5))
        sp = ctx.enter_context(tc.tile_pool(name="sp", bufs=6))
        pp = ctx.enter_context(tc.tile_pool(name="pp", bufs=4))
        xp = ctx.enter_context(tc.tile_pool(name="xp", bufs=4))
        ps = ctx.enter_context(tc.tile_pool(name="ps", bufs=8, space="PSUM"))
        dramp = ctx.enter_context(tc.tile_pool(name="dramp", bufs=1, space="DRAM"))

        ident = cst.tile([128, 128], F32R)
        nc.scalar.dma_start(ident, id_d)
        zeros = cst.tile([128, 512], F32R)
        nc.scalar.dma_start(zeros, zero_d)
        eps_t = cst.tile([128, 1], F32)
        nc.vector.memset(eps_t, 1e-5)

        # one DRAM scratch tile per derived matrix: dependency tracking is
        # then per-matrix, so a layer only waits for the matrix it reads.
        bmat_t = {k: dramp.tile([D, D], F32R, tag=f"bm{k}", name=f"bm{k}")
                  for k in need_b}
        cmat_t = {k: dramp.tile([D, D], F32R, tag=f"cm{k}", name=f"cm{k}")
                  for k in need_c}

        # block-0 x loads first: issued from the idle gpsimd engine
        next_x_tiles = []
        for r in range(RC):
            xt0 = xp.tile([128, D], F32R, tag="x", name=f"x0_{r}")
            nc.gpsimd.dma_start(xt0, x_d[r * 128:(r + 1) * 128, :])
            next_x_tiles.append(xt0)

        def precompute(kind, k):
            att, mult, dstm = ((As_d, t2s_d, bmat_t[k]) if kind == "b"
                               else (At_d, s2t_d, cmat_t[k]))
            # C_0 (gates layer 1) rides the (then idle) scalar ring; all
            # others ride the gpsimd SWDGE ring so the ACT engine never
            # spends ~620ns per dma_start in the middle of the LN pipeline
            eng = nc.scalar if (kind == "c" and k == 0) else nc.gpsimd
            for kc in range(KC):
                a_t = pp.tile([128, D], F32R, tag="pre", name="pa")
                eng.dma_start(a_t, att[k, kc * 128:(kc + 1) * 128, :])
                m_t = pp.tile([128, D], F32R, tag="pre", name="pm")
                eng.dma_start(m_t, mult[kc * 128:(kc + 1) * 128, :])
                d_t = pp.tile([128, D], F32R, tag="pre", name="pd")
                nc.gpsimd.tensor_mul(d_t, a_t, m_t)
                eng.dma_start(dstm[kc * 128:(kc + 1) * 128, :], d_t)

        # first-use layer 1-3 matrices up front
        if 0 in need_c:
            precompute("c", 0)
        if 0 in need_b:
            precompute("b", 0)
        if 2 in need_c:
            precompute("c", 2)

        def wsrc(i, j):
            k = i - j - 1
            if i % 2 == 0 and j % 2 == 0:
                return As_d[k]
            if i % 2 == 0:
                return bmat_t[k]
            if j % 2 == 0:
                return cmat_t[k]
            return At_d[k]

        for b in range(NBLK):
            hT = [hist.tile([128, KC, block], F32R, tag=f"hT{j}", name=f"hT{j}")
                  for j in range(L - 1)]

            def warm(n, y_tile, hT0=None):
                # zero-contribution matmuls: keep the PE array active (HAM
                # clock gate) across the LN/transpose dependency stall.
                src = hT0 if hT0 is not None else hT[0]
                for _ in range(n):
                    nc.tensor.matmul(y_tile, lhsT=src[:, 0, 0:128],
                                     rhs=zeros, start=False, stop=True,
                                     skip_group_check=True)

            def transpose_into(dst_hT, src_tile, r, halves=None):
                # src [128 rows, D] -> dst[:, dc, r*128:(r+1)*128] for all dc
                for half in (range(KC // 4) if halves is None else halves):
                    tp = ps.tile([128, 4, 128], F32R, tag="acc", name="tp")
                    for q in range(4):
                        dc = half * 4 + q
                        nc.tensor.transpose(
                            tp[:, q, :], src_tile[:, dc * 128:(dc + 1) * 128],
                            ident)
                    nc.scalar.copy(
                        dst_hT[:, half * 4:half * 4 + 4, r * 128:(r + 1) * 128],
                        tp)

            # history[0] = x (transposed into SBUF); tiles were prefetched
            # during the previous block
            for r in range(RC):
                transpose_into(hT[0], next_x_tiles[r], r)

            for i in range(1, L):
                # j order: middle js first, then the j consuming the
                # freshest DERIVED matrix (j=0 for odd i, j=1 for even i),
                # then the freshest-HISTORY j (i-1) last.
                jfresh = 0 if i % 2 == 1 else 1
                js = [j for j in range(i - 1) if j != jfresh]
                if jfresh < i - 1:
                    js.append(jfresh)
                js.append(i - 1)
                # the last TAILKC k-chunks of the last j run r-chunk-outer:
                # bank r0 then stops accumulating ~TAILKC*RC matmuls before
                # bank r3, so its relu/LN/transpose chain overlaps with the
                # remaining real matmuls instead of idling the PE at the
                # layer boundary.
                jlast = js[-1]
                head = [(j, kc) for j in js for kc in range(KC)
                        if not (j == jlast and kc >= KC - TAILKC)]
                y = [[ps.tile([128, 512], F32, tag="acc", name=f"y{r}_{c}")
                      for c in range(CC)] for r in range(RC)]
                for n, (j, kc) in enumerate(head):
                    w_t = wp.tile([128, D], F32R, tag="w", name="wt")
                    nc.sync.dma_start(w_t, wsrc(i, j)[kc * 128:(kc + 1) * 128, :])
                    for r in range(RC):
                        lhsT = hT[j][:, kc, r * 128:(r + 1) * 128]
                        for c in range(CC):
                            nc.tensor.matmul(
                                y[r][c], lhsT=lhsT,
                                rhs=w_t[:, c * 512:(c + 1) * 512],
                                start=(n == 0), stop=False)
                wtail = {}
                for kc in range(KC - TAILKC, KC):
                    w_t = wp.tile([128, D], F32R, tag="w", name="wtl")
                    nc.sync.dma_start(w_t, wsrc(i, jlast)[kc * 128:(kc + 1) * 128, :])
                    wtail[kc] = w_t
                for r in range(RC):
                    for kc in range(KC - TAILKC, KC):
                        lhsT = hT[jlast][:, kc, r * 128:(r + 1) * 128]
                        for c in range(CC):
                            nc.tensor.matmul(
                                y[r][c], lhsT=lhsT,
                                rhs=wtail[kc][:, c * 512:(c + 1) * 512],
                                start=False, stop=(kc == KC - 1))
                inv = 1.0 / i
                for r in range(RC):
                    z = zp.tile([128, D], F32, tag="z", name="z")
                    for c in range(CC):
                        nc.scalar.activation(z[:, c * 512:(c + 1) * 512],
                                             y[r][c], Relu, scale=inv)
                    st = sp.tile([128, CC, 6], F32, tag="st", name="st")
                    for c in range(CC):
                        nc.vector.bn_stats(st[:, c, :], z[:, c * 512:(c + 1) * 512])
                    mv = sp.tile([128, 2], F32, tag="mv", name="mv")
                    nc.vector.bn_aggr(mv, st)
                    rstd = sp.tile([128, 1], F32, tag="rs", name="rs")
                    nc.scalar.activation(rstd, mv[:, 1:2], Sqrt, bias=eps_t)
                    nc.vector.reciprocal(rstd, rstd)
                    h = hp.tile([128, D], F32R, tag="h", name="h")
                    nc.vector.tensor_scalar(
                        out=h, in0=z, scalar1=mv[:, 0:1], scalar2=rstd,
                        op0=mybir.AluOpType.subtract, op1=mybir.AluOpType.mult)
                    oi = i - (L - 2)
                    if oi >= 0:
                        row0 = b * block + r * 128
                        nc.gpsimd.dma_start(out_d[oi, row0:row0 + 128, :], h)
                    if i < L - 1:
                        transpose_into(hT[i], h, r)

                # block 0: emit remaining precomputes with ~2 layers of
                # slack before first consumption (C_2 was done up front)
                if b == 0:
                    if i == 1 and 2 in need_b:
                        precompute("b", 2)
                    elif i == 2 and 4 in need_c:
                        precompute("c", 4)
                    elif i == 3 and 4 in need_b:
                        precompute("b", 4)
                    elif i == 4 and 6 in need_c:
                        precompute("c", 6)

                # prefetch the next block's x tiles well ahead of their use
                if i == max(1, L - 2) and b + 1 < NBLK:
                    next_x_tiles = []
                    for r in range(RC):
                        xt = xp.tile([128, D], F32R, tag="x", name="xt")
                        row0 = (b + 1) * block + r * 128
                        nc.gpsimd.dma_start(xt, x_d[row0:row0 + 128, :])
                        next_x_tiles.append(xt)

    nc.compile()
    return nc


def kernel(x, source_to_target, target_to_source, att_source, att_target,
           num_layers):
    from concourse.bass_utils import run_bass_kernel_spmd

    x = np.ascontiguousarray(np.asarray(x, dtype=np.float32))
    s2t = np.ascontiguousarray(np.asarray(source_to_target, dtype=np.float32))
    t2s = np.ascontiguousarray(np.asarray(target_to_source, dtype=np.float32))
    As = np.ascontiguousarray(np.asarray(att_source, dtype=np.float32))
    At = np.ascontiguousarray(np.asarray(att_target, dtype=np.float32))
    L = int(num_layers)

    N, D = x.shape
    S = As.shape[0]
    n_cores = 8
    assert N % n_cores == 0
    rows = N // n_cores
    block = 512 if rows % 512 == 0 else 128

    key = (L, rows, D, S, block, n_cores)
    if key not in _CACHE:
        _CACHE[key] = _build(L, rows, D, S, block, n_cores)
    nc = _CACHE[key]

    ident = np.eye(128, dtype=np.float32)
    zeros = np.zeros((128, 512), dtype=np.float32)
    in_maps = [
        {
            "x": x[c * rows:(c + 1) * rows],
            "source_to_target": s2t,
            "target_to_source": t2s,
            "att_source": As,
            "att_target": At,
            "ident": ident,
            "zeros": zeros,
        }
        for c in range(n_cores)
    ]
    res = run_bass_kernel_spmd(nc, in_maps, list(range(n_cores))).results
    out = np.concatenate([res[c]["out"] for c in range(n_cores)], axis=1)
    if L == 2:
        out[0] = x  # history[-2] is the input itself
    return out.astype(np.float32, copy=False)



# revision 3
# speedup vs baseline: 1.1329x; 1.1329x over previous
"""Trainium2 Bass kernel for nn_BipartiteGraph1d (gnn_message_passing).

Reference computation (N=16384 rows, D=1024 features, L=num_layers=8):
    history[0] = x
    for i in 1..L-1:
        y = mean_j( history[j] @ m(i,j) )   j in 0..i-1, k = i-j-1
            m(i,j) = att_source[k]                    (i even, j even)
                     target_to_source * att_source[k] (i even, j odd)
                     source_to_target * att_target[k] (i odd,  j even)
                     att_target[k]                    (i odd,  j odd)
        history.append(layernorm(relu(y)))
    out = stack(history[-2:])                         (2, N, D)

Strategy (8 NeuronCores, data-parallel over rows), bf16 matmul path:
  * each core gets 2048 rows, processed in 512-row blocks whose full layer
    history lives in SBUF as PE-transposed BF16 tiles hT[j] = h_j.T.
  * ALL weight matrices (both the raw att stacks and the elementwise
    products with source_to_target / target_to_source) are converted once
    to BF16 DRAM scratch on-device, then streamed as [128, 1024] bf16
    chunks (half the HBM traffic of fp32; FWL halves LDWEIGHTS time).
    The end-to-end rel-err of the bf16 pipeline vs the fp32 reference is
    ~1.3e-2 (measured in emulation), within the 2e-2 gate.
  * per layer, the mean over j is accumulated directly in PSUM (fp32)
    across all (j, k-chunk) contributions: 8 psum banks = 4 row-chunks x
    2 dout-chunks.
  * derived matrices ride the DVE/gpsimd (multiply, bf16 out); direct
    matrices are converted by a single gpsimd cast-DMA (fp32 DRAM ->
    bf16 DRAM); emission is interleaved so each matrix lands in scratch
    just before the layer that first consumes it.
  * per layer the j's are ordered so the freshest derived matrix is
    second-to-last and the freshest history tile last - maximum slack for
    the LN->transpose pipeline and the precompute.
  * relu+layernorm run natively per-row (rows on partitions) on ACT/DVE in
    fp32; normalized output is PE-transposed (f32r) and converted to BF16
    at the psum->SBUF copy into the history.
"""

import numpy as np

_CACHE = {}
_LDW_PATCHED = False


def _enable_ldw_opt():
    """Walrus ships with --enable-ldw-opt=false; our matmul stream issues
    consecutive same-weight pairs (c=0/1 share lhsT), so redundant
    LDWEIGHTS elision is pure win. Correctness is covered by the
    end-to-end error check."""
    global _LDW_PATCHED
    if _LDW_PATCHED:
        return
    import concourse.bass_utils as bu
    orig = bu.run_command

    def patched(argv, **kw):
        argv = ["--enable-ldw-opt=true" if a == "--enable-ldw-opt=false" else a
                for a in argv]
        return orig(argv, **kw)

    bu.run_command = patched
    _LDW_PATCHED = True

TAILKC = 3       # k-chunks of the last j run r-outer (boundary overlap)


def _build(L, rows_per_core, D, S, block, num_devices):
    import concourse.tile as tile
    import concourse.mybir as mybir
    from concourse import bacc
    from contextlib import ExitStack

    F32R = mybir.dt.float32r
    F32 = mybir.dt.float32
    BF16 = mybir.dt.bfloat16
    Relu = mybir.ActivationFunctionType.Relu
    Sqrt = mybir.ActivationFunctionType.Sqrt

    assert D == 1024, "layout hardcodes D=1024"
    assert rows_per_core % block == 0 and block % 128 == 0
    assert 2 <= L <= S + 1
    KC = D // 128          # contraction chunks per matrix
    RC = block // 128      # row chunks per block
    CC = D // 512          # dout chunks (psum bank width)
    NBLK = rows_per_core // block

    nc = bacc.Bacc("TRN2", target_bir_lowering=False, debug=False,
                   num_devices=num_devices)
    x_d = nc.dram_tensor("x", [rows_per_core, D], F32R, kind="ExternalInput").ap()
    s2t_d = nc.dram_tensor("source_to_target", [D, D], F32R, kind="ExternalInput").ap()
    t2s_d = nc.dram_tensor("target_to_source", [D, D], F32R, kind="ExternalInput").ap()
    As_d = nc.dram_tensor("att_source", [S, D, D], F32R, kind="ExternalInput").ap()
    At_d = nc.dram_tensor("att_target", [S, D, D], F32R, kind="ExternalInput").ap()
    id_d = nc.dram_tensor("ident", [128, 128], F32R, kind="ExternalInput").ap()
    out_d = nc.dram_tensor("out", [2, rows_per_core, D], F32R,
                           kind="ExternalOutput").ap()

    # matrices needed (k = i-j-1, always < S here):
    #   derived b[k] = t2s * As[k]   (i even, j odd)   first use i = k+2
    #   derived c[k] = s2t * At[k]   (i odd,  j even)  first use i = k+1|k+2
    #   direct  As[k]                (i even, j even)  bf16 convert
    #   direct  At[k]                (i odd,  j odd)   bf16 convert
    need_b = sorted({i - j - 1 for i in range(1, L) for j in range(i)
                     if i % 2 == 0 and j % 2 == 1})
    need_c = sorted({i - j - 1 for i in range(1, L) for j in range(i)
                     if i % 2 == 1 and j % 2 == 0})
    need_s = sorted({i - j - 1 for i in range(1, L) for j in range(i)
                     if i % 2 == 0 and j % 2 == 0})
    need_t = sorted({i - j - 1 for i in range(1, L) for j in range(i)
                     if i % 2 == 1 and j % 2 == 1})

    def first_use(kind, k):
        for i in range(1, L):
            for j in range(i):
                if i - j - 1 != k:
                    continue
                kd = ("s" if j % 2 == 0 else "b") if i % 2 == 0 else \
                     ("c" if j % 2 == 0 else "t")
                if kd == kind:
                    return i
        return None

    # emission schedule: (kind, k) sorted by first use; emit each item
    # ~2 layers before its first consumer during block 0.
    items = ([("b", k) for k in need_b] + [("c", k) for k in need_c] +
             [("s", k) for k in need_s] + [("t", k) for k in need_t])
    items = [(kind, k, first_use(kind, k)) for kind, k in items]
    items.sort(key=lambda it: (it[2], it[0] in ("s", "t")))
    upfront = [(kind, k) for kind, k, fu in items if fu <= 2]
    emit_at = {}
    for kind, k, fu in items:
        if fu > 2:
            emit_at.setdefault(max(1, fu - 2), []).append((kind, k))

    with tile.TileContext(nc) as tc, ExitStack() as ctx:
        cst = ctx.enter_context(tc.tile_pool(name="cst", bufs=1))
        hist = ctx.enter_context(tc.tile_pool(name="hist", bufs=1))
        wp = ctx.enter_context(tc.tile_pool(name="wp", bufs=8))
        zp = ctx.enter_context(tc.tile_pool(name="zp", bufs=3))
        hp = ctx.enter_context(tc.tile_pool(name="hp", bufs=5))
        sp = ctx.enter_context(tc.tile_pool(name="sp", bufs=6))
        pp = ctx.enter_context(tc.tile_pool(name="pp", bufs=6))
        xp = ctx.enter_context(tc.tile_pool(name="xp", bufs=4))
        ps = ctx.enter_context(tc.tile_pool(name="ps", bufs=8, space="PSUM"))
        dramp = ctx.enter_context(tc.tile_pool(name="dramp", bufs=1, space="DRAM"))

        ident = cst.tile([128, 128], F32R)
        nc.scalar.dma_start(ident, id_d)
        eps_t = cst.tile([128, 1], F32)
        nc.vector.memset(eps_t, 1e-5)

        # one BF16 DRAM scratch tile per matrix: dependency tracking is
        # per-matrix, so a layer only waits for the matrix it reads.
        mat_t = {}
        for k in need_b:
            mat_t["b", k] = dramp.tile([D, D], BF16, tag=f"bm{k}", name=f"bm{k}")
        for k in need_c:
            mat_t["c", k] = dramp.tile([D, D], BF16, tag=f"cm{k}", name=f"cm{k}")
        for k in need_s:
            mat_t["s", k] = dramp.tile([D, D], BF16, tag=f"sm{k}", name=f"sm{k}")
        for k in need_t:
            mat_t["t", k] = dramp.tile([D, D], BF16, tag=f"tm{k}", name=f"tm{k}")

        # block-0 x loads first: issued from the idle gpsimd engine
        next_x_tiles = []
        for r in range(RC):
            xt0 = xp.tile([128, D], F32R, tag="x", name=f"x0_{r}")
            nc.gpsimd.dma_start(xt0, x_d[r * 128:(r + 1) * 128, :])
            next_x_tiles.append(xt0)

        def precompute(kind, k):
            dstm = mat_t[kind, k]
            if kind in ("s", "t"):
                # direct att matrix: fp32 -> bf16 via gpsimd cast-DMA,
                # DRAM -> DRAM, no compute engine involved.
                att = As_d if kind == "s" else At_d
                for kc in range(0, KC, 2):
                    nc.gpsimd.dma_start(
                        dstm[kc * 128:(kc + 2) * 128, :],
                        att[k, kc * 128:(kc + 2) * 128, :])
                return
            att, mult = (As_d, t2s_d) if kind == "b" else (At_d, s2t_d)
            # C_0 (gates layer 1) rides the (then idle) scalar ring; all
            # others ride the gpsimd SWDGE ring so the ACT engine never
            # spends ~620ns per dma_start in the middle of the LN pipeline
            eng = nc.scalar if (kind == "c" and k == 0) else nc.gpsimd
            for kc in range(KC):
                a_t = pp.tile([128, D], F32R, tag="pre", name="pa")
                eng.dma_start(a_t, att[k, kc * 128:(kc + 1) * 128, :])
                m_t = pp.tile([128, D], F32R, tag="pre", name="pm")
                eng.dma_start(m_t, mult[kc * 128:(kc + 1) * 128, :])
                d_t = pp.tile([128, D], BF16, tag="pre", name="pd")
                nc.gpsimd.tensor_mul(d_t, a_t, m_t)
                eng.dma_start(dstm[kc * 128:(kc + 1) * 128, :], d_t)

        for kind, k in upfront:
            precompute(kind, k)

        def wsrc(i, j):
            k = i - j - 1
            if i % 2 == 0 and j % 2 == 0:
                return mat_t["s", k]
            if i % 2 == 0:
                return mat_t["b", k]
            if j % 2 == 0:
                return mat_t["c", k]
            return mat_t["t", k]

        for b in range(NBLK):
            hT = [hist.tile([128, KC, block], BF16, tag=f"hT{j}", name=f"hT{j}")
                  for j in range(L - 1)]

            def transpose_into(dst_hT, src_tile, r, halves=None):
                # src [128 rows, D] -> dst[:, dc, r*128:(r+1)*128] for all dc
                for half in (range(KC // 4) if halves is None else halves):
                    tp = ps.tile([128, 4, 128], F32R, tag="acc", name="tp")
                    for q in range(4):
                        dc = half * 4 + q
                        nc.tensor.transpose(
                            tp[:, q, :], src_tile[:, dc * 128:(dc + 1) * 128],
                            ident)
                    nc.scalar.copy(
                        dst_hT[:, half * 4:half * 4 + 4, r * 128:(r + 1) * 128],
                        tp)

            # history[0] = x (transposed into SBUF); tiles were prefetched
            # during the previous block
            for r in range(RC):
                transpose_into(hT[0], next_x_tiles[r], r)

            for i in range(1, L):
                # j order: middle js first, then the j consuming the
                # freshest DERIVED matrix (j=0 for odd i, j=1 for even i),
                # then the freshest-HISTORY j (i-1) last.
                jfresh = 0 if i % 2 == 1 else 1
                js = [j for j in range(i - 1) if j != jfresh]
                if jfresh < i - 1:
                    js.append(jfresh)
                js.append(i - 1)
                # the last TAILKC k-chunks of the last j run r-chunk-outer:
                # bank r0 then stops accumulating ~TAILKC*RC matmuls before
                # bank r3, so its relu/LN/transpose chain overlaps with the
                # remaining real matmuls instead of idling the PE at the
                # layer boundary.
                jlast = js[-1]
                head = [(j, kc) for j in js for kc in range(KC)
                        if not (j == jlast and kc >= KC - TAILKC)]
                y = [[ps.tile([128, 512], F32, tag="acc", name=f"y{r}_{c}")
                      for c in range(CC)] for r in range(RC)]
                for n, (j, kc) in enumerate(head):
                    w_t = wp.tile([128, D], BF16, tag="w", name="wt")
                    nc.sync.dma_start(w_t, wsrc(i, j)[kc * 128:(kc + 1) * 128, :])
                    for r in range(RC):
                        lhsT = hT[j][:, kc, r * 128:(r + 1) * 128]
                        for c in range(CC):
                            nc.tensor.matmul(
                                y[r][c], lhsT=lhsT,
                                rhs=w_t[:, c * 512:(c + 1) * 512],
                                start=(n == 0), stop=False)
                wtail = {}
                for kc in range(KC - TAILKC, KC):
                    w_t = wp.tile([128, D], BF16, tag="w", name="wtl")
                    nc.sync.dma_start(w_t, wsrc(i, jlast)[kc * 128:(kc + 1) * 128, :])
                    wtail[kc] = w_t
                for r in range(RC):
                    for kc in range(KC - TAILKC, KC):
                        lhsT = hT[jlast][:, kc, r * 128:(r + 1) * 128]
                        for c in range(CC):
                            nc.tensor.matmul(
                                y[r][c], lhsT=lhsT,
                                rhs=wtail[kc][:, c * 512:(c + 1) * 512],
                                start=False, stop=(kc == KC - 1))
                inv = 1.0 / i
                for r in range(RC):
                    z = zp.tile([128, D], F32, tag="z", name="z")
                    for c in range(CC):
                        nc.scalar.activation(z[:, c * 512:(c + 1) * 512],
                                             y[r][c], Relu, scale=inv)
                    st = sp.tile([128, CC, 6], F32, tag="st", name="st")
                    for c in range(CC):
                        nc.vector.bn_stats(st[:, c, :], z[:, c * 512:(c + 1) * 512])
                    mv = sp.tile([128, 2], F32, tag="mv", name="mv")
                    nc.vector.bn_aggr(mv, st)
                    rstd = sp.tile([128, 1], F32, tag="rs", name="rs")
                    nc.scalar.activation(rstd, mv[:, 1:2], Sqrt, bias=eps_t)
                    nc.vector.reciprocal(rstd, rstd)
                    h = hp.tile([128, D], F32R, tag="h", name="h")
                    nc.vector.tensor_scalar(
                        out=h, in0=z, scalar1=mv[:, 0:1], scalar2=rstd,
                        op0=mybir.AluOpType.subtract, op1=mybir.AluOpType.mult)
                    oi = i - (L - 2)
                    if oi >= 0:
                        row0 = b * block + r * 128
                        nc.gpsimd.dma_start(out_d[oi, row0:row0 + 128, :], h)
                    if i < L - 1:
                        transpose_into(hT[i], h, r)

                # block 0: emit remaining precomputes with ~2 layers of
                # slack before first consumption
                if b == 0:
                    for kind, k in emit_at.get(i, []):
                        precompute(kind, k)

                # prefetch the next block's x tiles well ahead of their use
                if i == max(1, L - 2) and b + 1 < NBLK:
                    next_x_tiles = []
                    for r in range(RC):
                        xt = xp.tile([128, D], F32R, tag="x", name="xt")
                        row0 = (b + 1) * block + r * 128
                        nc.gpsimd.dma_start(xt, x_d[row0:row0 + 128, :])
                        next_x_tiles.append(xt)

    nc.compile()
    return nc


def kernel(x, source_to_target, target_to_source, att_source, att_target,
           num_layers):
    from concourse.bass_utils import run_bass_kernel_spmd

    x = np.ascontiguousarray(np.asarray(x, dtype=np.float32))
    s2t = np.ascontiguousarray(np.asarray(source_to_target, dtype=np.float32))
    t2s = np.ascontiguousarray(np.asarray(target_to_source, dtype=np.float32))
    As = np.ascontiguousarray(np.asarray(att_source, dtype=np.float32))
    At = np.ascontiguousarray(np.asarray(att_target, dtype=np.float32))
    L = int(num_layers)

    N, D = x.shape
    S = As.shape[0]
    n_cores = 8
    assert N % n_cores == 0
    rows = N // n_cores
    block = 512 if rows % 512 == 0 else 128

    key = (L, rows, D, S, block, n_cores)
    if key not in _CACHE:
        _CACHE[key] = _build(L, rows, D, S, block, n_cores)
    nc = _CACHE[key]

    ident = np.eye(128, dtype=np.float32)
    in_maps = [
        {
            "x": x[c * rows:(c + 1) * rows],
            "source_to_target": s2t,
            "target_to_source": t2s,
            "att_source": As,
            "att_target": At,
            "ident": ident,
        }
        for c in range(n_cores)
    ]
    res = run_bass_kernel_spmd(nc, in_maps, list(range(n_cores))).results
    out = np.concatenate([res[c]["out"] for c in range(n_cores)], axis=1)
    if L == 2:
        out[0] = x  # history[-2] is the input itself
    return out.astype(np.float32, copy=False)


# revision 5
# speedup vs baseline: 1.2407x; 1.0951x over previous
"""Trainium2 Bass kernel for nn_BipartiteGraph1d (gnn_message_passing).

Reference computation (N=16384 rows, D=1024 features, L=num_layers=8):
    history[0] = x
    for i in 1..L-1:
        y = mean_j( history[j] @ m(i,j) )   j in 0..i-1, k = i-j-1
            m(i,j) = att_source[k]                    (i even, j even)
                     target_to_source * att_source[k] (i even, j odd)
                     source_to_target * att_target[k] (i odd,  j even)
                     att_target[k]                    (i odd,  j odd)
        history.append(layernorm(relu(y)))
    out = stack(history[-2:])                         (2, N, D)

Strategy (8 NeuronCores, data-parallel over rows), bf16 matmul path:
  * each core gets 2048 rows, processed in 512-row blocks whose full layer
    history lives in SBUF as PE-transposed BF16 tiles hT[j] = h_j.T.
  * ALL weight matrices (att stacks and their elementwise products with
    source_to_target / target_to_source) are used in BF16: half the HBM
    traffic of fp32 and FWL halves LDWEIGHTS. End-to-end rel-err of the
    bf16 pipeline vs the fp32 reference is ~1.3e-2 (within the 2e-2 gate).
  * block 0 streams each matrix's FIRST use straight from the fp32
    sources: direct matrices ride a gpsimd cast-DMA (fp32->bf16) into the
    weight tile; derived matrices are cast-loaded and multiplied on the
    DVE with an SBUF-cached bf16 copy of source_to_target /
    target_to_source, writing the BF16 DRAM scratch as a side effect.
    Later uses (and blocks 1+) stream the scratch. This keeps the PE fed
    from t=0 instead of stalling ~250us on a precompute round trip.
  * per layer, the mean over j is accumulated directly in PSUM (fp32)
    across all (j, k-chunk) contributions: 8 psum banks = 4 row-chunks x
    2 dout-chunks; weight chunks ride the sync-engine HWDGE ring.
  * per layer the j's are ordered so the freshest derived matrix is
    second-to-last and the freshest history tile last - maximum slack for
    the LN->transpose pipeline.
  * relu+layernorm run natively per-row (rows on partitions) on ACT/DVE in
    fp32; normalized output is PE-transposed (f32r) and converted to BF16
    at the psum->SBUF copy into the history.
"""

import numpy as np

_CACHE = {}
_LDW_PATCHED = False


def _enable_ldw_opt():
    """Walrus ships with --enable-ldw-opt=false; our matmul stream issues
    consecutive same-weight pairs (c=0/1 share lhsT), so redundant
    LDWEIGHTS elision is pure win. Correctness is covered by the
    end-to-end error check."""
    global _LDW_PATCHED
    if _LDW_PATCHED:
        return
    import concourse.bass_utils as bu
    orig = bu.run_command

    def patched(argv, **kw):
        argv = ["--enable-ldw-opt=true" if a == "--enable-ldw-opt=false" else a
                for a in argv]
        return orig(argv, **kw)

    bu.run_command = patched
    _LDW_PATCHED = True

TAILKC = 3       # k-chunks of the last j run r-outer (boundary overlap)


def _build(L, rows_per_core, D, S, block, num_devices):
    import concourse.tile as tile
    import concourse.mybir as mybir
    from concourse import bacc
    from contextlib import ExitStack

    F32R = mybir.dt.float32r
    F32 = mybir.dt.float32
    BF16 = mybir.dt.bfloat16
    Relu = mybir.ActivationFunctionType.Relu
    Sqrt = mybir.ActivationFunctionType.Sqrt

    assert D == 1024, "layout hardcodes D=1024"
    assert rows_per_core % block == 0 and block % 128 == 0
    assert 2 <= L <= S + 1
    KC = D // 128          # contraction chunks per matrix
    RC = block // 128      # row chunks per block
    CC = D // 512          # dout chunks (psum bank width)
    NBLK = rows_per_core // block

    nc = bacc.Bacc("TRN2", target_bir_lowering=False, debug=False,
                   num_devices=num_devices)
    x_d = nc.dram_tensor("x", [rows_per_core, D], F32R, kind="ExternalInput").ap()
    s2t_d = nc.dram_tensor("source_to_target", [D, D], F32R, kind="ExternalInput").ap()
    t2s_d = nc.dram_tensor("target_to_source", [D, D], F32R, kind="ExternalInput").ap()
    As_d = nc.dram_tensor("att_source", [S, D, D], F32R, kind="ExternalInput").ap()
    At_d = nc.dram_tensor("att_target", [S, D, D], F32R, kind="ExternalInput").ap()
    id_d = nc.dram_tensor("ident", [128, 128], F32R, kind="ExternalInput").ap()
    out_d = nc.dram_tensor("out", [2, rows_per_core, D], F32R,
                           kind="ExternalOutput").ap()

    # matrix kinds (k = i-j-1, always < S here):
    #   derived b[k] = t2s * As[k]   (i even, j odd)
    #   derived c[k] = s2t * At[k]   (i odd,  j even)
    #   direct  As[k]                (i even, j even)
    #   direct  At[k]                (i odd,  j odd)
    def mkind(i, j):
        if i % 2 == 0:
            return "s" if j % 2 == 0 else "b"
        return "c" if j % 2 == 0 else "t"

    need = {}
    for i in range(1, L):
        for j in range(i):
            kd, k = mkind(i, j), i - j - 1
            need.setdefault((kd, k), i)   # records first-use layer

    with tile.TileContext(nc) as tc, ExitStack() as ctx:
        cst = ctx.enter_context(tc.tile_pool(name="cst", bufs=1))
        hist = ctx.enter_context(tc.tile_pool(name="hist", bufs=1))
        wp = ctx.enter_context(tc.tile_pool(name="wp", bufs=8))
        zp = ctx.enter_context(tc.tile_pool(name="zp", bufs=3))
        hp = ctx.enter_context(tc.tile_pool(name="hp", bufs=5))
        sp = ctx.enter_context(tc.tile_pool(name="sp", bufs=6))
        pp = ctx.enter_context(tc.tile_pool(name="pp", bufs=6))
        xp = ctx.enter_context(tc.tile_pool(name="xp", bufs=4))
        ps = ctx.enter_context(tc.tile_pool(name="ps", bufs=8, space="PSUM"))
        dramp = ctx.enter_context(tc.tile_pool(name="dramp", bufs=1, space="DRAM"))

        ident = cst.tile([128, 128], F32R)
        nc.scalar.dma_start(ident, id_d)
        eps_t = cst.tile([128, 1], F32)
        nc.vector.memset(eps_t, 1e-5)

        # block-0 x loads first: issued from the idle gpsimd engine
        next_x_tiles = []
        for r in range(RC):
            xt0 = xp.tile([128, D], F32R, tag="x", name=f"x0_{r}")
            nc.gpsimd.dma_start(xt0, x_d[r * 128:(r + 1) * 128, :])
            next_x_tiles.append(xt0)

        # SBUF-cached bf16 copies of the two multiplier matrices
        multb = {}
        if any(kd == "c" for kd, _ in need):
            multb["c"] = cst.tile([128, KC, D], BF16, name="s2tb")
        if any(kd == "b" for kd, _ in need):
            multb["b"] = cst.tile([128, KC, D], BF16, name="t2sb")
        for kd, src in (("c", s2t_d), ("b", t2s_d)):
            if kd in multb:
                for kc in range(KC):
                    nc.gpsimd.dma_start(multb[kd][:, kc, :],
                                        src[kc * 128:(kc + 1) * 128, :])

        # one BF16 DRAM scratch tile per matrix: dependency tracking is
        # per-matrix, so a layer only waits for the matrix it reads.
        mat_t = {key: dramp.tile([D, D], BF16, tag=f"{key[0]}m{key[1]}",
                                 name=f"{key[0]}m{key[1]}")
                 for key in need}

        # direct matrices also need a DRAM->DRAM cast conversion for their
        # later (scratch) uses; emit it one layer after the first use so it
        # never contends with the first-use stream on the gpsimd ring.
        emit_at = {}
        for (kd, k), fu in need.items():
            if kd in ("s", "t"):
                emit_at.setdefault(min(fu + 1, L - 1), []).append((kd, k))

        def convert_direct(kd, k):
            att = As_d if kd == "s" else At_d
            dstm = mat_t[kd, k]
            for kc in range(0, KC, 2):
                nc.gpsimd.dma_start(
                    dstm[kc * 128:(kc + 2) * 128, :],
                    att[k, kc * 128:(kc + 2) * 128, :])

        def load_w(i, j, kc, b):
            """Weight chunk [128, D] bf16 for (i, j), contraction chunk kc."""
            kd, k = mkind(i, j), i - j - 1
            w_t = wp.tile([128, D], BF16, tag="w", name="wt")
            if b == 0 and need[kd, k] == i:
                att = As_d if kd in ("s", "b") else At_d
                if kd in ("s", "t"):
                    # first use of a direct matrix: cast-DMA from fp32
                    nc.gpsimd.dma_start(w_t, att[k, kc * 128:(kc + 1) * 128, :])
                else:
                    # first use of a derived matrix: cast-load the att
                    # chunk, multiply on DVE, and write scratch on the side
                    a_t = pp.tile([128, D], BF16, tag="pre", name="pa")
                    nc.gpsimd.dma_start(a_t, att[k, kc * 128:(kc + 1) * 128, :])
                    nc.vector.tensor_mul(w_t, a_t, multb[kd][:, kc, :])
                    nc.gpsimd.dma_start(mat_t[kd, k][kc * 128:(kc + 1) * 128, :],
                                        w_t)
            else:
                nc.sync.dma_start(w_t, mat_t[kd, k][kc * 128:(kc + 1) * 128, :])
            return w_t

        for b in range(NBLK):
            hT = [hist.tile([128, KC, block], BF16, tag=f"hT{j}", name=f"hT{j}")
                  for j in range(L - 1)]

            def transpose_into(dst_hT, src_tile, r, halves=None):
                # src [128 rows, D] -> dst[:, dc, r*128:(r+1)*128] for all dc
                for half in (range(KC // 4) if halves is None else halves):
                    tp = ps.tile([128, 4, 128], F32R, tag="acc", name="tp")
                    for q in range(4):
                        dc = half * 4 + q
                        nc.tensor.transpose(
                            tp[:, q, :], src_tile[:, dc * 128:(dc + 1) * 128],
                            ident)
                    nc.scalar.copy(
                        dst_hT[:, half * 4:half * 4 + 4, r * 128:(r + 1) * 128],
                        tp)

            # history[0] = x (transposed into SBUF); tiles were prefetched
            # during the previous block
            for r in range(RC):
                transpose_into(hT[0], next_x_tiles[r], r)

            for i in range(1, L):
                # j order: middle js first, then the j consuming the
                # freshest DERIVED matrix (j=0 for odd i, j=1 for even i),
                # then the freshest-HISTORY j (i-1) last.
                jfresh = 0 if i % 2 == 1 else 1
                js = [j for j in range(i - 1) if j != jfresh]
                if jfresh < i - 1:
                    js.append(jfresh)
                js.append(i - 1)
                # the last TAILKC k-chunks of the last j run r-chunk-outer:
                # bank r0 then stops accumulating ~TAILKC*RC matmuls before
                # bank r3, so its relu/LN/transpose chain overlaps with the
                # remaining real matmuls instead of idling the PE at the
                # layer boundary.
                jlast = js[-1]
                head = [(j, kc) for j in js for kc in range(KC)
                        if not (j == jlast and kc >= KC - TAILKC)]
                y = [[ps.tile([128, 512], F32, tag="acc", name=f"y{r}_{c}")
                      for c in range(CC)] for r in range(RC)]
                for n, (j, kc) in enumerate(head):
                    w_t = load_w(i, j, kc, b)
                    for r in range(RC):
                        lhsT = hT[j][:, kc, r * 128:(r + 1) * 128]
                        for c in range(CC):
                            nc.tensor.matmul(
                                y[r][c], lhsT=lhsT,
                                rhs=w_t[:, c * 512:(c + 1) * 512],
                                start=(n == 0), stop=False)
                wtail = {}
                for kc in range(KC - TAILKC, KC):
                    wtail[kc] = load_w(i, jlast, kc, b)
                for r in range(RC):
                    for kc in range(KC - TAILKC, KC):
                        lhsT = hT[jlast][:, kc, r * 128:(r + 1) * 128]
                        for c in range(CC):
                            nc.tensor.matmul(
                                y[r][c], lhsT=lhsT,
                                rhs=wtail[kc][:, c * 512:(c + 1) * 512],
                                start=False, stop=(kc == KC - 1))
                inv = 1.0 / i
                for r in range(RC):
                    z = zp.tile([128, D], F32, tag="z", name="z")
                    for c in range(CC):
                        nc.scalar.activation(z[:, c * 512:(c + 1) * 512],
                                             y[r][c], Relu, scale=inv)
                    st = sp.tile([128, CC, 6], F32, tag="st", name="st")
                    for c in range(CC):
                        nc.vector.bn_stats(st[:, c, :], z[:, c * 512:(c + 1) * 512])
                    mv = sp.tile([128, 2], F32, tag="mv", name="mv")
                    nc.vector.bn_aggr(mv, st)
                    rstd = sp.tile([128, 1], F32, tag="rs", name="rs")
                    nc.scalar.activation(rstd, mv[:, 1:2], Sqrt, bias=eps_t)
                    nc.vector.reciprocal(rstd, rstd)
                    h = hp.tile([128, D], F32R, tag="h", name="h")
                    nc.vector.tensor_scalar(
                        out=h, in0=z, scalar1=mv[:, 0:1], scalar2=rstd,
                        op0=mybir.AluOpType.subtract, op1=mybir.AluOpType.mult)
                    oi = i - (L - 2)
                    if oi >= 0:
                        row0 = b * block + r * 128
                        nc.gpsimd.dma_start(out_d[oi, row0:row0 + 128, :], h)
                    if i < L - 1:
                        transpose_into(hT[i], h, r)

                # block 0: DRAM->DRAM bf16 conversion of direct matrices
                # for their later (scratch-fed) uses
                if b == 0:
                    for kd, k in emit_at.get(i, []):
                        convert_direct(kd, k)

                # prefetch the next block's x tiles well ahead of their use
                if i == max(1, L - 2) and b + 1 < NBLK:
                    next_x_tiles = []
                    for r in range(RC):
                        xt = xp.tile([128, D], F32R, tag="x", name="xt")
                        row0 = (b + 1) * block + r * 128
                        nc.gpsimd.dma_start(xt, x_d[row0:row0 + 128, :])
                        next_x_tiles.append(xt)

    nc.compile()
    return nc


def kernel(x, source_to_target, target_to_source, att_source, att_target,
           num_layers):
    from concourse.bass_utils import run_bass_kernel_spmd

    x = np.ascontiguousarray(np.asarray(x, dtype=np.float32))
    s2t = np.ascontiguousarray(np.asarray(source_to_target, dtype=np.float32))
    t2s = np.ascontiguousarray(np.asarray(target_to_source, dtype=np.float32))
    As = np.ascontiguousarray(np.asarray(att_source, dtype=np.float32))
    At = np.ascontiguousarray(np.asarray(att_target, dtype=np.float32))
    L = int(num_layers)

    N, D = x.shape
    S = As.shape[0]
    n_cores = 8
    assert N % n_cores == 0
    rows = N // n_cores
    block = 512 if rows % 512 == 0 else 128

    key = (L, rows, D, S, block, n_cores)
    if key not in _CACHE:
        _CACHE[key] = _build(L, rows, D, S, block, n_cores)
    nc = _CACHE[key]

    ident = np.eye(128, dtype=np.float32)
    in_maps = [
        {
            "x": x[c * rows:(c + 1) * rows],
            "source_to_target": s2t,
            "target_to_source": t2s,
            "att_source": As,
            "att_target": At,
            "ident": ident,
        }
        for c in range(n_cores)
    ]
    res = run_bass_kernel_spmd(nc, in_maps, list(range(n_cores))).results
    out = np.concatenate([res[c]["out"] for c in range(n_cores)], axis=1)
    if L == 2:
        out[0] = x  # history[-2] is the input itself
    return out.astype(np.float32, copy=False)


# revision 13
# speedup vs baseline: 1.3703x; 1.1045x over previous
"""Trainium2 Bass kernel for nn_BipartiteGraph1d (gnn_message_passing).

Reference computation (N=16384 rows, D=1024 features, L=num_layers=8):
    history[0] = x
    for i in 1..L-1:
        y = mean_j( history[j] @ m(i,j) )   j in 0..i-1, k = i-j-1
            m(i,j) = att_source[k]                    (i even, j even)
                     target_to_source * att_source[k] (i even, j odd)
                     source_to_target * att_target[k] (i odd,  j even)
                     att_target[k]                    (i odd,  j odd)
        history.append(layernorm(relu(y)))
    out = stack(history[-2:])                         (2, N, D)

Strategy (8 NeuronCores, data-parallel over rows), bf16 matmul path:
  * each core gets 2048 rows, processed in 512-row blocks whose full layer
    history lives in SBUF as PE-transposed BF16 tiles hT[j] = h_j.T.
  * ALL weight matrices (att stacks and their products with
    source_to_target / target_to_source) are used in BF16: half the HBM
    traffic of fp32, FWL halves LDWEIGHTS. End-to-end rel-err of the bf16
    pipeline vs the fp32 reference is ~1.3e-2 (within the 2e-2 gate).
  * block 0 streams each matrix's FIRST use straight from the fp32
    sources (gpsimd cast-DMA for the raw att matrices; f32 load + DVE
    multiply against an SBUF-cached bf16 multiplier for the derived
    ones), writing the BF16 DRAM scratch as a side effect. Later uses and
    blocks 1+ stream the scratch. The PE is fed from t=0; no precompute
    round trip on the critical path.
  * each layer accumulates in TWO half-width passes (dout 0:512, 512:1024)
    of 4 PSUM banks each, drawn from a 6-buffer rotation; the other 2
    banks are a dedicated pool for PE-transpose outputs. With the
    rotation, a pass never waits on a bank that was not already drained.
  * layer i's LN outputs are transposed into the history by PE matmuls
    that are INTERLEAVED into layer i+1's weight-chunk stream (and the
    next block's x transposes into layer L-1), so the serial
    relu->stats->normalize chain hides under real matmuls instead of
    stalling the in-order PE queue at every layer boundary.
  * relu+layernorm run natively per-row on ACT/DVE in fp32; the LN output
    is written in BF16 (fp32 only for the two output layers) and
    transposed in bf16 (half the PE cost of f32r transposes).
"""

import numpy as np

_CACHE = {}

TAILKC = 3       # k-chunks of the last j run r-outer (boundary overlap)


def _build(L, rows_per_core, D, S, block, num_devices):
    import concourse.tile as tile
    import concourse.mybir as mybir
    from concourse import bacc
    from contextlib import ExitStack

    F32R = mybir.dt.float32r
    F32 = mybir.dt.float32
    BF16 = mybir.dt.bfloat16
    Relu = mybir.ActivationFunctionType.Relu
    Sqrt = mybir.ActivationFunctionType.Sqrt

    assert D == 1024, "layout hardcodes D=1024"
    assert rows_per_core % block == 0 and block % 128 == 0
    assert 2 <= L <= S + 1
    KC = D // 128          # contraction chunks per matrix
    RC = block // 128      # row chunks per block
    NBLK = rows_per_core // block
    HW = 512               # half-width pass (one psum bank)

    nc = bacc.Bacc("TRN2", target_bir_lowering=False, debug=False,
                   num_devices=num_devices)
    x_d = nc.dram_tensor("x", [rows_per_core, D], F32R, kind="ExternalInput").ap()
    s2t_d = nc.dram_tensor("source_to_target", [D, D], F32R, kind="ExternalInput").ap()
    t2s_d = nc.dram_tensor("target_to_source", [D, D], F32R, kind="ExternalInput").ap()
    As_d = nc.dram_tensor("att_source", [S, D, D], F32R, kind="ExternalInput").ap()
    At_d = nc.dram_tensor("att_target", [S, D, D], F32R, kind="ExternalInput").ap()
    id_d = nc.dram_tensor("ident", [128, 128], F32R, kind="ExternalInput").ap()
    out_d = nc.dram_tensor("out", [2, rows_per_core, D], F32R,
                           kind="ExternalOutput").ap()

    # matrix kinds (k = i-j-1, always < S here):
    #   derived b[k] = t2s * As[k]   (i even, j odd)
    #   derived c[k] = s2t * At[k]   (i odd,  j even)
    #   direct  As[k]                (i even, j even)
    #   direct  At[k]                (i odd,  j odd)
    def mkind(i, j):
        if i % 2 == 0:
            return "s" if j % 2 == 0 else "b"
        return "c" if j % 2 == 0 else "t"

    need = {}
    for i in range(1, L):
        for j in range(i):
            need.setdefault((mkind(i, j), i - j - 1), i)   # first-use layer

    with tile.TileContext(nc) as tc, ExitStack() as ctx:
        cst = ctx.enter_context(tc.tile_pool(name="cst", bufs=1))
        hist = ctx.enter_context(tc.tile_pool(name="hist", bufs=1))
        wp = ctx.enter_context(tc.tile_pool(name="wp", bufs=12))
        zp = ctx.enter_context(tc.tile_pool(name="zp", bufs=6))
        hp = ctx.enter_context(tc.tile_pool(name="hp", bufs=3))
        hpb = ctx.enter_context(tc.tile_pool(name="hpb", bufs=6))
        sp = ctx.enter_context(tc.tile_pool(name="sp", bufs=6))
        pp = ctx.enter_context(tc.tile_pool(name="pp", bufs=6))
        xp = ctx.enter_context(tc.tile_pool(name="xp", bufs=4))
        ps = ctx.enter_context(tc.tile_pool(name="ps", bufs=7, space="PSUM"))
        tpp = ctx.enter_context(tc.tile_pool(name="tpp", bufs=1, space="PSUM"))
        dramp = ctx.enter_context(tc.tile_pool(name="dramp", bufs=1, space="DRAM"))

        identb = cst.tile([128, 128], BF16)
        nc.gpsimd.dma_start(identb, id_d)        # cast f32 -> bf16
        eps_t = cst.tile([128, 1], F32)
        nc.vector.memset(eps_t, 1e-5)

        # block-0 x loads first (cast to bf16 on the gpsimd SWDGE ring)
        next_x_tiles = []
        for r in range(RC):
            xt0 = xp.tile([128, D], BF16, tag="x", name=f"x0_{r}")
            nc.gpsimd.dma_start(xt0, x_d[r * 128:(r + 1) * 128, :])
            next_x_tiles.append(xt0)

        # SBUF-cached bf16 copies of the two multiplier matrices
        multb = {}
        for kd, src in (("c", s2t_d), ("b", t2s_d)):
            if any(kd2 == kd for kd2, _ in need):
                mt = cst.tile([128, KC, D], BF16, name=f"mult{kd}")
                for kc in range(KC):
                    nc.gpsimd.dma_start(mt[:, kc, :],
                                        src[kc * 128:(kc + 1) * 128, :])
                multb[kd] = mt

        # one BF16 DRAM scratch tile per matrix: dependency tracking is
        # per-matrix, so a layer only waits for the matrix it reads.
        mat_t = {key: dramp.tile([D, D], BF16, tag=f"{key[0]}m{key[1]}",
                                 name=f"{key[0]}m{key[1]}")
                 for key in need}

        # direct matrices also need a DRAM->DRAM cast conversion for their
        # later (scratch) uses; emit it one layer after the first use so it
        # never contends with the first-use stream on the gpsimd ring.
        emit_at = {}
        for (kd, k), fu in need.items():
            if kd in ("s", "t"):
                emit_at.setdefault(min(fu + 1, L - 1), []).append((kd, k))

        def convert_direct(kd, k):
            att = As_d if kd == "s" else At_d
            dstm = mat_t[kd, k]
            for kc in range(0, KC, 2):
                nc.gpsimd.dma_start(
                    dstm[kc * 128:(kc + 2) * 128, :],
                    att[k, kc * 128:(kc + 2) * 128, :])

        def load_w(i, j, kc, half, b):
            """Weight chunk [128, HW] bf16 for (i, j), chunk kc, dout half."""
            kd, k = mkind(i, j), i - j - 1
            cols = slice(half * HW, (half + 1) * HW)
            w_t = wp.tile([128, HW], BF16, tag="w", name="wt")
            if b == 0 and need[kd, k] == i:
                att = As_d if kd in ("s", "b") else At_d
                src = att[k, kc * 128:(kc + 1) * 128, cols]
                if kd in ("s", "t"):
                    # first use of a direct matrix: cast-DMA from fp32
                    nc.gpsimd.dma_start(w_t, src)
                else:
                    # first use of a derived matrix: f32 load on the (idle)
                    # scalar HWDGE ring, DVE multiply (bf16 out), and write
                    # the scratch on the side via gpsimd
                    a_t = pp.tile([128, HW], F32R, tag="pre", name="pa")
                    nc.scalar.dma_start(a_t, src)
                    nc.vector.tensor_mul(w_t, a_t, multb[kd][:, kc, cols])
                    nc.gpsimd.dma_start(mat_t[kd, k][kc * 128:(kc + 1) * 128, cols],
                                        w_t)
            else:
                nc.sync.dma_start(w_t, mat_t[kd, k][kc * 128:(kc + 1) * 128, cols])
            return w_t

        hT0_next = None
        for b in range(NBLK):
            if hT0_next is not None:
                hT = [hT0_next]
            else:
                hT = [hist.tile([128, KC, block], BF16, tag="hT0", name="hT0")]
            hT += [hist.tile([128, KC, block], BF16, tag=f"hT{j}", name=f"hT{j}")
                   for j in range(1, L - 1)]
            hT0_next = None

            def tp_group(dst_hT, src_tile, r, half):
                # PE-transpose src[:, half*512:(half+1)*512] bf16 into
                # dst[:, dc, r*128:(r+1)*128] for 4 dc, via one psum tile
                tp = tpp.tile([128, 4, 128], BF16, tag="tp", name="tp")
                for q in range(4):
                    dc = half * 4 + q
                    nc.tensor.transpose(
                        tp[:, q, :], src_tile[:, dc * 128:(dc + 1) * 128],
                        identb)
                nc.scalar.copy(
                    dst_hT[:, half * 4:half * 4 + 4, r * 128:(r + 1) * 128],
                    tp)

            def tp_groups(dst_hT, src_tile, r):
                for half in range(KC // 4):
                    tp_group(dst_hT, src_tile, r, half)

            # block-0 (and L==2, where layer L-1 cannot host the
            # interleave): x transposes happen right here
            if b == 0 or L == 2:
                for r in range(RC):
                    tp_groups(hT[0], next_x_tiles[r], r)

            pending_tp = []    # [(dst_hT, src_bf16_tile, r)] from layer i-1
            for i in range(1, L):
                if b == 0:
                    # freshest DERIVED matrix second-to-last, freshest
                    # history last - slack for first-use streams.
                    jfresh = 0 if i % 2 == 1 else 1
                    js = [j for j in range(i - 1) if j != jfresh]
                    if jfresh < i - 1:
                        js.append(jfresh)
                    js.append(i - 1)
                else:
                    js = list(range(i))
                jlast = js[-1]
                head = [(j, kc) for j in js for kc in range(KC)
                        if not (j == jlast and kc >= KC - TAILKC)]

                # interleave schedule for deferred transposes (they only
                # depend on layer i-1's LN outputs / prefetched x tiles);
                # one 4-dc group per slot so consecutive groups never wait
                # on each other's psum-evacuation copy
                # pending_tp writes hT[i-1], which THIS layer's jlast
                # chunks read - every group must be emitted strictly
                # before jlast's first head chunk (position cap), else the
                # reads are emitted first and consume stale history.
                jlast_start = (len(js) - 1) * KC
                tp_sched = {}
                for idx, (dst, src, r2) in enumerate(pending_tp):
                    for g in range(2):
                        pos = min(2 + 3 * idx + g, jlast_start - 1,
                                  len(head) - 1)
                        tp_sched.setdefault(pos, []).append((dst, src, r2, g))
                # next block's x transposes reuse the hT0 BUFFER (tag
                # rotation, bufs=1) while the old hT0 tile still has
                # readers in this layer's LATER emission - so they must be
                # interleaved into the LAST pass (half==1), after its j=0
                # chunks; every old-hT0 read is emitted before them then.
                tp_sched_p1 = {}
                if i == L - 1 and b + 1 < NBLK and L > 2:
                    j0_end = (js.index(0) + 1) * KC
                    hT0_next = hist.tile([128, KC, block], BF16, tag="hT0",
                                         name="hT0")
                    for idx, r2 in enumerate(range(RC)):
                        for g in range(2):
                            pos = min(j0_end + 1 + 3 * idx + g, len(head) - 1)
                            tp_sched_p1.setdefault(pos, []).append(
                                (hT0_next, next_x_tiles[r2], r2, g))

                z = [None] * RC
                for half in range(2):
                    y = [ps.tile([128, HW], F32, tag="acc", name=f"y{r}")
                         for r in range(RC)]
                    for n, (j, kc) in enumerate(head):
                        w_t = load_w(i, j, kc, half, b)
                        for r in range(RC):
                            nc.tensor.matmul(
                                y[r], lhsT=hT[j][:, kc, r * 128:(r + 1) * 128],
                                rhs=w_t, start=(n == 0), stop=False)
                        sched = tp_sched if half == 0 else tp_sched_p1
                        for dst, src, r2, g in sched.get(n, []):
                            tp_group(dst, src, r2, g)
                    wtail = {kc: load_w(i, jlast, kc, half, b)
                             for kc in range(KC - TAILKC, KC)}
                    for r in range(RC):
                        for kc in range(KC - TAILKC, KC):
                            nc.tensor.matmul(
                                y[r], lhsT=hT[jlast][:, kc, r * 128:(r + 1) * 128],
                                rhs=wtail[kc], start=False, stop=(kc == KC - 1))
                    inv = 1.0 / i
                    for r in range(RC):
                        if half == 0:
                            z[r] = zp.tile([128, D], F32, tag="z", name="z")
                        nc.scalar.activation(z[r][:, half * HW:(half + 1) * HW],
                                             y[r], Relu, scale=inv)

                pending_tp = []
                for r in range(RC):
                    st = sp.tile([128, 2, 6], F32, tag="st", name="st")
                    for c in range(2):
                        nc.vector.bn_stats(st[:, c, :], z[r][:, c * HW:(c + 1) * HW])
                    mv = sp.tile([128, 2], F32, tag="mv", name="mv")
                    nc.vector.bn_aggr(mv, st)
                    rstd = sp.tile([128, 1], F32, tag="rs", name="rs")
                    nc.scalar.activation(rstd, mv[:, 1:2], Sqrt, bias=eps_t)
                    nc.vector.reciprocal(rstd, rstd)
                    oi = i - (L - 2)
                    if oi >= 0:
                        h32 = hp.tile([128, D], F32R, tag="h32", name="h32")
                        nc.vector.tensor_scalar(
                            out=h32, in0=z[r], scalar1=mv[:, 0:1], scalar2=rstd,
                            op0=mybir.AluOpType.subtract, op1=mybir.AluOpType.mult)
                        row0 = b * block + r * 128
                        nc.gpsimd.dma_start(out_d[oi, row0:row0 + 128, :], h32)
                        if i < L - 1:
                            h_bf = hpb.tile([128, D], BF16, tag="hb", name="hb")
                            nc.vector.tensor_copy(h_bf, h32)
                            pending_tp.append((hT[i], h_bf, r))
                    else:
                        h_bf = hpb.tile([128, D], BF16, tag="hb", name="hb")
                        nc.vector.tensor_scalar(
                            out=h_bf, in0=z[r], scalar1=mv[:, 0:1], scalar2=rstd,
                            op0=mybir.AluOpType.subtract, op1=mybir.AluOpType.mult)
                        pending_tp.append((hT[i], h_bf, r))

                # block 0: DRAM->DRAM bf16 conversion of direct matrices
                # for their later (scratch-fed) uses
                if b == 0:
                    for kd, k in emit_at.get(i, []):
                        convert_direct(kd, k)

                # prefetch the next block's x tiles well ahead of their use
                if i == max(1, L - 2) and b + 1 < NBLK:
                    next_x_tiles = []
                    for r in range(RC):
                        xt = xp.tile([128, D], BF16, tag="x", name="xt")
                        row0 = (b + 1) * block + r * 128
                        nc.gpsimd.dma_start(xt, x_d[row0:row0 + 128, :])
                        next_x_tiles.append(xt)

    nc.compile()
    return nc


def kernel(x, source_to_target, target_to_source, att_source, att_target,
           num_layers):
    from concourse.bass_utils import run_bass_kernel_spmd

    x = np.ascontiguousarray(np.asarray(x, dtype=np.float32))
    s2t = np.ascontiguousarray(np.asarray(source_to_target, dtype=np.float32))
    t2s = np.ascontiguousarray(np.asarray(target_to_source, dtype=np.float32))
    As = np.ascontiguousarray(np.asarray(att_source, dtype=np.float32))
    At = np.ascontiguousarray(np.asarray(att_target, dtype=np.float32))
    L = int(num_layers)

    N, D = x.shape
    S = As.shape[0]
    n_cores = 8
    assert N % n_cores == 0
    rows = N // n_cores
    block = 512 if rows % 512 == 0 else 128

    key = (L, rows, D, S, block, n_cores)
    if key not in _CACHE:
        _CACHE[key] = _build(L, rows, D, S, block, n_cores)
    nc = _CACHE[key]

    ident = np.eye(128, dtype=np.float32)
    in_maps = [
        {
            "x": x[c * rows:(c + 1) * rows],
            "source_to_target": s2t,
            "target_to_source": t2s,
            "att_source": As,
            "att_target": At,
            "ident": ident,
        }
        for c in range(n_cores)
    ]
    res = run_bass_kernel_spmd(nc, in_maps, list(range(n_cores))).results
    out = np.concatenate([res[c]["out"] for c in range(n_cores)], axis=1)
    if L == 2:
        out[0] = x  # history[-2] is the input itself
    return out.astype(np.float32, copy=False)
